# revision 15
# baseline (speedup 1.0000x reference)
"""Trainium2 Bass kernel for nn_MultiHeadAttention_7541962572058.

Self-contained: hardcodes shapes B=32, N=512, D=512, H=8, DEG=8 and the
canonical edge list from the problem's setup_inputs() (node i connects to
(i+1..i+8) mod N within its batch; edge e = (b*N + i)*DEG + (d-1)).

Sharding: data-parallel over batch across 8 NeuronCores (4 batches/core);
weights replicated; edge tensors sharded by batch. No collectives.

Device-side plan (per core, per batch; validated against the reference by a
numpy prototype):
  - activations kept feature-major (x.T) via PE transposes
  - q.T/k.T = W.T @ x.T (weights stationary); v token-major
  - edge MLP over slot-major edge columns e' = s*512 + i, edge features
    loaded pre-transposed via bf16 DMA-transpose
  - local scores via shifted-view DVE muls + ones-select PE reduction
  - local/global softmax row-major, probs scattered to a dense matrix with
    gpsimd local_scatter, PE-transposed, then P.T used in AV matmuls
  - out.T = Wo.T @ ctx.T ; edge-update MLP reuses out.T columns through
    shifted views (no gather), with the unshifted half A1 computed once
  - SSP offset (softplus(x) - ln2) folded into the following layer's bias
"""
import math

import numpy as np
import ml_dtypes

import concourse.bass as bass
import concourse.mybir as mybir
import concourse.tile as tile
from concourse import bacc
from concourse.bass_utils import run_bass_kernel_spmd

F32 = mybir.dt.float32
F32R = mybir.dt.float32r
BF16 = mybir.dt.float16
I16 = mybir.dt.int16
AF = mybir.ActivationFunctionType
AX = mybir.AxisListType
ALU = mybir.AluOpType

B, N, D, H = 32, 512, 512, 8
DEG = 8
DH = D // H       # 64
H2 = H // 2       # 4
LN2 = math.log(2.0)
NB = 4            # batches per core
NCORES = 8
PT = 128          # partitions
KT = D // PT      # 4 k-tiles over D
ITILES = N // PT  # 4 token tiles

_BF = np.float16

_PROGRAM_CACHE = {}


def _seg_pairs(delta, n=N):
    """dst col ranges and src col ranges for src[(c + delta) % n]."""
    if delta == 0:
        return [((0, n), (0, n))]
    return [((0, n - delta), (delta, n)), ((n - delta, n), (0, delta))]


def _emit(tc, t):
    nc = tc.nc
    import contextlib
    ctx = contextlib.ExitStack()

    pool = lambda name, bufs, space="SBUF": ctx.enter_context(
        tc.tile_pool(name=name, bufs=bufs, space=space))

    wpool = pool("weights", 1)
    xTp = pool("xT", 3)
    qkp = pool("qk", 1)
    vtokp = pool("vtok", 1)
    eftp = pool("eft", 2)
    efts_p = pool("efts", 1)
    h1p = pool("h1", 4)
    slp = pool("slsb", 1)
    alp = pool("al", 2)
    pp = pool("praw", 1)
    pgp = pool("pg", 1)
    ptp = pool("pt", 2)
    invp = pool("invm", 1)
    ctxp = pool("ctxT", 1)
    outTp = pool("outT", 1)
    otokp = pool("otok", 2)
    a1p = pool("a1", 1)
    hsp = pool("hs", 1)
    usbp = pool("usb", 2)
    topp = pool("top", 1)
    smallp = pool("small", 2)

    pmm = pool("pmm", 3, "PSUM")
    ptr = pool("ptr", 2, "PSUM")
    psl = pool("psl", 1, "PSUM")
    pav = pool("pav", 2, "PSUM")

    # ---------------- persistent weights/constants ----------------
    def wload(name, dram, shape, dt_):
        s = wpool.tile(shape, dt_, tag=name)
        nc.sync.dma_start(s[:], dram)
        return s

    wq = wload("wq", t["wq"].rearrange("(kt p) n -> p kt n", p=PT), [PT, KT, D], F32R)
    wk = wload("wk", t["wk"].rearrange("(kt p) n -> p kt n", p=PT), [PT, KT, D], F32R)
    wv = wload("wv", t["wv"].rearrange("(kt p) n -> p kt n", p=PT), [PT, KT, D], F32R)
    wo = wload("wo", t["wo"].rearrange("(kt p) n -> p kt n", p=PT), [PT, KT, D], F32R)
    euw1 = wload("euw1", t["euw1"].rearrange("(kt p) n -> p kt n", p=PT),
                 [PT, 2 * KT, D], BF16)
    epw1 = wload("epw1", t["epw1"].rearrange("(kt p) n -> p kt n", p=PT),
                 [PT, KT, D], BF16)
    epw2 = wload("epw2", t["epw2"].rearrange("(kt p) n -> p kt n", p=PT),
                 [PT, KT, D // 2], BF16)
    euw2 = wload("euw2", t["euw2"].rearrange("(kt p) n -> p kt n", p=PT),
                 [PT, KT, D], BF16)

    # per-partition biases [128, mt]
    bq8 = wload("bq8", t["bq8"].rearrange("(mt p) -> p mt", p=PT), [PT, KT], F32)
    bk = wload("bk", t["bk"].rearrange("(mt p) -> p mt", p=PT), [PT, KT], F32)
    bo = wload("bo", t["bo"].rearrange("(mt p) -> p mt", p=PT), [PT, KT], F32)
    epb1 = wload("epb1", t["epb1"].rearrange("(mt p) -> p mt", p=PT), [PT, KT], F32)
    epb2a = wload("epb2a", t["epb2a"].rearrange("(mt p) -> p mt", p=PT),
                  [PT, D // 2 // PT], F32)
    eub1 = wload("eub1", t["eub1"].rearrange("(mt p) -> p mt", p=PT), [PT, KT], F32)
    # row-layout biases [1, n] for K=1 matmuls
    bvr = wload("bvr", t["bv"].bitcast(F32R).rearrange("(a n) -> a n", a=1), [1, D], F32R)
    eub2r = wload("eub2r", t["eub2a"].bitcast(F32R).rearrange("(a n) -> a n", a=1), [1, D], F32R)

    onesel = wload("onesel", t["onesel"].rearrange("s h p c -> p s h c"),
                   [PT, DEG, 2, 32], F32R)
    scidx = wload("scidx", t["scidx"].rearrange("it p c -> p it c"),
                  [PT, ITILES, 16], I16)
    identf = wload("identf", t["identf"][:], [PT, PT], F32)
    identr = wload("identr", t["identr"][:], [PT, PT], F32R)
    identb = wload("identb", t["identb"][:], [PT, PT], BF16)

    ones_row = wload("ones_row", t["onesr"][:], [1, D], F32R)
    halfb = wpool.tile([PT, 1], F32, tag="halfb")
    nc.vector.memset(halfb[:], 0.5)

    copy_flip = [0]

    def pcopy(dst, src):
        # alternate psum->sbuf copies between DVE and ACT
        if copy_flip[0] % 2 == 0:
            nc.vector.tensor_copy(dst, src)
        else:
            nc.scalar.activation(dst, src, AF.Identity, bias=0.0)
        copy_flip[0] += 1

    # ---------------- per batch ----------------
    for b in range(NB):
        # ---- P1: load x.T (host pre-transposed, feature-major) ----
        xT = {}
        for name, dram in (("q", t["xq"]), ("k", t["xk"]), ("v", t["xv"])):
            xt = xTp.tile([PT, KT, N], F32R, tag="xT")
            nc.sync.dma_start(
                xt[:], dram[b].rearrange("(kt p) n -> p kt n", p=PT))
            xT[name] = xt

        # ---- P2: projections ----
        qT = qkp.tile([PT, KT, N], F32R, tag="qT")
        kT = qkp.tile([PT, KT, N], F32R, tag="kT")
        for dst, w, xt, bias_, scale in (
                (qT, wq, xT["q"], bq8, 0.125), (kT, wk, xT["k"], bk, 1.0)):
            for mt in range(KT):
                ps = pmm.tile([PT, N], F32, tag="pmm")
                for kt in range(KT):
                    nc.tensor.matmul(
                        ps[:], w[:, kt, mt * PT:(mt + 1) * PT],
                        xt[:, kt, :],
                        start=(kt == 0), stop=(kt == KT - 1))
                nc.scalar.activation(dst[:, mt, :], ps[:], AF.Identity,
                                     bias=bias_[:, mt:mt + 1], scale=scale)

        vtok = vtokp.tile([PT, ITILES, D], BF16, tag="vtok")
        for tg in range(ITILES):
            ps = pmm.tile([PT, D], F32, tag="pmm")
            for kt in range(KT):
                nc.tensor.matmul(
                    ps[:], xT["v"][:, kt, tg * PT:(tg + 1) * PT],
                    wv[:, kt, :], start=(kt == 0), stop=False)
            nc.tensor.matmul(ps[:], ones_row[:, 0:PT], bvr[:],
                             start=False, stop=True)
            nc.vector.tensor_copy(vtok[:, tg, :], ps[:])

        # ---- P3+P4: edge MLP (slot-major) and local scores ----
        sl_ps = psl.tile([32, N], F32, tag="psl")
        for s in range(DEG):
            eft = eftp.tile([PT, KT, N], BF16, tag="eft")
            nc.sync.dma_start(
                eft[:], t["efsm"][b, s].rearrange("(di p) i -> p di i", p=PT))
            h1 = []
            for mt in range(KT):
                ps = pmm.tile([PT, N], F32, tag="pmm")
                for kt in range(KT):
                    nc.tensor.matmul(ps[:],
                                     epw1[:, kt, mt * PT:(mt + 1) * PT],
                                     eft[:, kt, :],
                                     start=(kt == 0), stop=(kt == KT - 1))
                ex = smallp.tile([PT, N], F32, tag="spexp")
                nc.scalar.activation(ex[:], ps[:], AF.Exp,
                                     bias=epb1[:, mt:mt + 1])
                h1t = h1p.tile([PT, N], BF16, tag="h1")
                nc.scalar.activation(h1t[:], ex[:], AF.Ln, bias=halfb[:], scale=0.5)
                h1.append(h1t)
            efts = efts_p.tile([PT, 2, N], F32, tag="efts")
            for fm in range(2):
                ps = pmm.tile([PT, N], F32, tag="pmm")
                for kt in range(KT):
                    nc.tensor.matmul(ps[:],
                                     epw2[:, kt, fm * PT:(fm + 1) * PT],
                                     h1[kt][:],
                                     start=(kt == 0), stop=(kt == KT - 1))
                nc.scalar.activation(efts[:, fm, :], ps[:], AF.Identity,
                                     bias=epb2a[:, fm:fm + 1])

            # local scores for slot s: rows f = h2*64+d ; kT/qT local = rows 256:512
            for hf in range(2):   # f-halves: mt = 2 + hf of kT/qT
                mt = 2 + hf
                sl3 = smallp.tile([PT, N], F32R, tag="sl3")
                for (d0, d1), (s0, s1) in _seg_pairs(s + 1):
                    nc.vector.tensor_tensor(
                        sl3[:, d0:d1], qT[:, mt, d0:d1], kT[:, mt, s0:s1],
                        op=ALU.mult)
                nc.vector.tensor_tensor(sl3[:], sl3[:], efts[:, hf, :],
                                        op=ALU.mult)
                nc.tensor.matmul(sl_ps[:], onesel[:, s, hf, :],
                                 sl3[:],
                                 start=(s == 0 and hf == 0),
                                 stop=(s == DEG - 1 and hf == 1))

        # ---- P5: local softmax -> al [128 i, (h2, s)] normalized bf16 ----
        slsb = slp.tile([32, N], F32R, tag="slsb")
        nc.scalar.activation(slsb[:], sl_ps[:], AF.Identity, bias=0.0)
        al_bf = {}
        for it in range(ITILES):
            pal = ptr.tile([PT, 32], F32R, tag="ptr")
            nc.tensor.transpose(pal[:], slsb[:, it * PT:(it + 1) * PT],
                                identr[0:32, 0:32])
            alsb = smallp.tile([PT, 32], F32, tag="alsb")
            nc.vector.tensor_copy(alsb[:], pal[:].bitcast(F32))
            v3 = alsb[:].rearrange("p (h s) -> p h s", h=H2)
            mx = smallp.tile([PT, H2], F32, tag="mx")
            nc.vector.tensor_reduce(mx[:], v3, axis=AX.X, op=ALU.max, negate=True)
            esb = smallp.tile([PT, 32], F32, tag="esb")
            nc.vector.tensor_tensor(esb[:].rearrange("p (h s) -> p h s", h=H2),
                                    v3, mx[:].broadcast_to([PT, H2, DEG]),
                                    op=ALU.add)
            esb2 = smallp.tile([PT, 32], F32, tag="esb2")
            nc.scalar.activation(esb2[:], esb[:], AF.Exp)
            sm = smallp.tile([PT, H2], F32, tag="sm")
            nc.vector.tensor_reduce(sm[:], esb2[:].rearrange("p (h s) -> p h s", h=H2),
                                    axis=AX.X, op=ALU.add)
            rc = smallp.tile([PT, H2], F32, tag="rc")
            nc.vector.reciprocal(rc[:], sm[:])
            alb = alp.tile([PT, 32], BF16, tag="al")
            nc.vector.tensor_tensor(alb[:].rearrange("p (h s) -> p h s", h=H2),
                                    esb2[:].rearrange("p (h s) -> p h s", h=H2),
                                    rc[:].broadcast_to([PT, H2, DEG]),
                                    op=ALU.mult)
            al_bf[it] = alb

        # ---- P6: scatter local probs to dense P[i, j] per head-pair ----
        P_loc = pp.tile([PT, ITILES, H2, N], BF16, tag="praw")
        for hp in range(2):
            for it in range(ITILES):
                nc.gpsimd.local_scatter(
                    P_loc[:, it, 2 * hp:2 * hp + 2, :],
                    al_bf[it][:, hp * 16:(hp + 1) * 16],
                    scidx[:, it, :], channels=PT, num_elems=2 * N, num_idxs=16)

        # ---- P8: global attention + P9 local AV -> ctxT ----
        invm = invp.tile([PT, ITILES, N], BF16, tag="invm")
        nc.sync.dma_start(invm[:],
                          t["invm"][b].rearrange("(it p) j -> p it j", p=PT))
        ctxT = ctxp.tile([PT, KT, N], F32R, tag="ctxT")

        for hg in range(H2):
            mt, prow = hg // 2, (hg % 2) * 64
            Pg = pgp.tile([PT, ITILES, N], BF16, tag="pg")
            for it in range(ITILES):
                ps = pmm.tile([PT, N], F32, tag="pmm")
                nc.tensor.matmul(
                    ps[:],
                    qT[prow:prow + 64, mt, it * PT:(it + 1) * PT],
                    kT[prow:prow + 64, mt, :],
                    start=True, stop=True)
                if hg == 0:
                    tops = topp.tile([PT, N], F32, tag="top")
                    nc.vector.tensor_copy(tops[:], ps[:])
                    nc.sync.dma_start(t["top"][b, it * PT:(it + 1) * PT, :],
                                      tops[:])
                mx = smallp.tile([PT, 1], F32, tag="gmx")
                nc.vector.tensor_reduce(mx[:], ps[:], axis=AX.X, op=ALU.max,
                                        negate=True)
                esb = smallp.tile([PT, N], BF16, tag="gesb")
                nc.scalar.activation(esb[:], ps[:], AF.Exp, bias=mx[:])
                e2 = smallp.tile([PT, N], BF16, tag="ge2")
                nc.vector.tensor_tensor(e2[:], esb[:], invm[:, it, :], op=ALU.mult)
                sm = smallp.tile([PT, 1], F32, tag="gsm")
                nc.vector.tensor_reduce(sm[:], e2[:], axis=AX.X, op=ALU.add)
                rc = smallp.tile([PT, 1], F32, tag="grc")
                nc.vector.reciprocal(rc[:], sm[:])
                nc.vector.tensor_scalar_mul(Pg[:, it, :], e2[:], rc[:])

            # transpose Pg -> PTg and AV
            ptg = ptp.tile([PT, ITILES, N], BF16, tag="pt")
            for it in range(ITILES):
                for jt in range(ITILES):
                    pt_ = ptr.tile([PT, PT], BF16, tag="ptr")
                    nc.tensor.transpose(pt_[:], Pg[:, it, jt * PT:(jt + 1) * PT],
                                        identb[:])
                    pcopy(ptg[:, jt, it * PT:(it + 1) * PT], pt_[:])
            av = pav.tile([64, N], F32, tag="pav")
            for jg in range(ITILES):
                nc.tensor.matmul(av[:], vtok[:, jg, hg * 64:(hg + 1) * 64],
                                 ptg[:, jg, :],
                                 start=(jg == 0), stop=(jg == ITILES - 1))
            nc.scalar.activation(ctxT[0:64, hg, :], av[:], AF.Identity, bias=0.0)

        for h2 in range(H2):
            ptl = ptp.tile([PT, ITILES, N], BF16, tag="pt")
            for it in range(ITILES):
                for jt in range(ITILES):
                    pt_ = ptr.tile([PT, PT], BF16, tag="ptr")
                    nc.tensor.transpose(
                        pt_[:], P_loc[:, it, h2, jt * PT:(jt + 1) * PT],
                        identb[:])
                    pcopy(ptl[:, jt, it * PT:(it + 1) * PT], pt_[:])
            av = pav.tile([64, N], F32, tag="pav")
            for jg in range(ITILES):
                nc.tensor.matmul(
                    av[:], vtok[:, jg, 256 + h2 * 64:256 + (h2 + 1) * 64],
                    ptl[:, jg, :], start=(jg == 0), stop=(jg == ITILES - 1))
            nc.scalar.activation(ctxT[64:128, h2, :], av[:], AF.Identity, bias=0.0)

        # ---- P10: out.T = Wo.T @ ctxT + bo ; out tokens to DRAM ----
        outT = outTp.tile([PT, KT, N], F32R, tag="outT")
        for mt in range(KT):
            ps = pmm.tile([PT, N], F32, tag="pmm")
            for kt in range(KT):
                nc.tensor.matmul(ps[:], wo[:, kt, mt * PT:(mt + 1) * PT],
                                 ctxT[:, kt, :],
                                 start=(kt == 0), stop=(kt == KT - 1))
            nc.scalar.activation(outT[:, mt, :], ps[:], AF.Identity,
                                 bias=bo[:, mt:mt + 1])
        outTb = outTp.tile([PT, KT, N], BF16, tag="outTb")
        for mt in range(KT):
            nc.vector.tensor_copy(outTb[:, mt, :], outT[:, mt, :])
        for tg in range(ITILES):
            otok = otokp.tile([PT, D], F32, tag="otok")
            for mt in range(KT):
                pt_ = ptr.tile([PT, PT], F32R, tag="ptr")
                nc.tensor.transpose(pt_[:], outT[:, mt, tg * PT:(tg + 1) * PT],
                                    identr[:])
                pcopy(otok[:, mt * PT:(mt + 1) * PT], pt_[:])
            nc.sync.dma_start(t["out"][b, tg * PT:(tg + 1) * PT, :], otok[:])

        # ---- P11: edge-update MLP ----
        a1 = a1p.tile([PT, KT, N], BF16, tag="a1")
        for mt in range(KT):
            ps = pmm.tile([PT, N], F32, tag="pmm")
            for kt in range(KT):
                nc.tensor.matmul(ps[:],
                                 euw1[:, kt, mt * PT:(mt + 1) * PT],
                                 outTb[:, kt, :],
                                 start=(kt == 0), stop=(kt == KT - 1))
            nc.scalar.activation(a1[:, mt, :], ps[:], AF.Identity,
                                 bias=eub1[:, mt:mt + 1])

        eupd_v = t["eupd"].rearrange("(bb et p s) d -> bb et s p d",
                                     bb=NB, et=ITILES, p=PT, s=DEG)
        for s in range(DEG):
            hs = hsp.tile([PT, KT, N], BF16, tag="hs")
            for mt in range(KT):
                ps = pmm.tile([PT, N], F32, tag="pmm")
                nc.tensor.matmul(ps[:], identb[:],
                                 a1[:, mt, :],
                                 start=True, stop=False)
                for kt in range(KT):
                    for (d0, d1), (s0, s1) in _seg_pairs(s + 1):
                        nc.tensor.matmul(
                            ps[:, d0:d1],
                            euw1[:, KT + kt, mt * PT:(mt + 1) * PT],
                            outTb[:, kt, s0:s1],
                            start=False,
                            stop=(kt == KT - 1 and d1 == N))
                ex = smallp.tile([PT, N], F32, tag="spexp")
                nc.scalar.activation(ex[:], ps[:], AF.Exp)
                nc.scalar.activation(hs[:, mt, :], ex[:], AF.Ln, bias=halfb[:], scale=0.5)
            for et in range(ITILES):
                ps = pmm.tile([PT, D], F32, tag="pmm")
                for kt in range(KT):
                    nc.tensor.matmul(ps[:], hs[:, kt, et * PT:(et + 1) * PT],
                                     euw2[:, kt, :], start=(kt == 0), stop=False)
                nc.tensor.matmul(ps[:], ones_row[:, 0:PT],
                                 eub2r[:], start=False, stop=True)
                usb = usbp.tile([PT, D], F32, tag="usb")
                pcopy(usb[:], ps[:])
                nc.sync.dma_start(eupd_v[b, et, s], usb[:])

    ctx.close()


def _patch_act_tables():
    """All activations used here (Exp, Ln, Identity, Copy) live together in
    the natural_log_exp_and_others set. The default per-instruction chooser
    picks the first containing table, which ping-pongs Exp->set0 / Ln->set5
    and inserts ~440 table reloads (~590us of ACT time). Present every other
    set as empty so the chooser lands on the combined set for everything;
    set ids keep their positions so walrus's act_info.json indexing is
    unchanged.
    """
    import concourse.hw_specs as hw_specs
    orig = hw_specs.get_activation_tables

    def patched(arch):
        tabs = orig(arch)
        keep = "natural_log_exp_and_others"
        assert keep in tabs
        return {name: (funcs if name == keep else set())
                for name, funcs in tabs.items()}

    bacc.get_activation_tables = patched


def _build_program():
    if "nc" in _PROGRAM_CACHE:
        return _PROGRAM_CACHE["nc"]
    _patch_act_tables()
    nc = bacc.Bacc("TRN2", target_bir_lowering=False, debug=False)
    t = {}
    di = lambda name, shape, dt_: t.__setitem__(
        name, nc.dram_tensor(name, shape, dt_, kind="ExternalInput")[:])
    do = lambda name, shape, dt_: t.__setitem__(
        name, nc.dram_tensor(name, shape, dt_, kind="ExternalOutput")[:])

    di("xk", [NB, D, N], F32R)
    di("xq", [NB, D, N], F32R)
    di("xv", [NB, D, N], F32R)
    di("efsm", [NB, DEG, D, N], BF16)
    di("invm", [NB, N, N], BF16)
    di("wq", [D, D], F32R); di("wk", [D, D], F32R)
    di("wv", [D, D], F32R); di("wo", [D, D], F32R)
    di("epw1", [D, D], BF16); di("epw2", [D, D // 2], BF16)
    di("euw1", [2 * D, D], BF16); di("euw2", [D, D], BF16)
    di("bq8", [D], F32); di("bk", [D], F32); di("bv", [D], F32); di("bo", [D], F32)
    di("epb1", [D], F32); di("epb2a", [D // 2], F32)
    di("eub1", [D], F32); di("eub2a", [D], F32)
    di("onesel", [DEG, 2, PT, 32], F32R)
    di("scidx", [ITILES, PT, 16], I16)
    di("identf", [PT, PT], F32)
    di("identr", [PT, PT], F32R)
    di("identb", [PT, PT], BF16)
    di("onesr", [1, D], F32R)

    do("out", [NB, N, D], F32)
    do("top", [NB, N, N], F32)
    do("eupd", [NB * N * DEG, D], F32)

    with tile.TileContext(nc) as tc:
        _emit(tc, t)
    nc.compile()
    _PROGRAM_CACHE["nc"] = nc
    return nc


def _host_constants():
    onesel = np.zeros((DEG, 2, PT, 32), np.float32)
    for s in range(DEG):
        for hf in range(2):
            for hloc in range(2):       # head within this f-half
                h2 = hf * 2 + hloc
                onesel[s, hf, hloc * 64:(hloc + 1) * 64, h2 * DEG + s] = 1.0
    scidx = np.zeros((ITILES, PT, 16), np.int16)
    for it in range(ITILES):
        for p in range(PT):
            gi = it * PT + p
            for hin in range(2):
                for s in range(DEG):
                    scidx[it, p, hin * DEG + s] = hin * N + (gi + 1 + s) % N
    identf = np.eye(PT, dtype=np.float32)
    identb = np.eye(PT, dtype=np.float32).astype(_BF)
    return onesel, scidx, identf, identb


def _canonical(pi):
    bi = np.repeat(np.arange(B), N * DEG)
    ii = np.tile(np.repeat(np.arange(N), DEG), B)
    jj = (ii + np.tile(np.arange(1, DEG + 1), B * N)) % N
    pi = np.asarray(pi)
    return (np.array_equal(pi[0], bi) and np.array_equal(pi[1], ii)
            and np.array_equal(pi[2], jj))


def _reference_numpy(inputs):
    """General fallback replicating the reference in numpy (slow path)."""
    f32 = lambda k: np.asarray(inputs[k], np.float32)
    key, value, query = f32("key"), f32("value"), f32("query")
    mask = np.asarray(inputs["mask"])
    ef, pi = f32("edge_feature"), np.asarray(inputs["pair_indices"])
    sp = lambda x: np.log1p(np.exp(-np.abs(x))) + np.maximum(x, 0) - LN2
    shape = lambda x: x.reshape(B, N, H, DH).transpose(0, 2, 1, 3)
    k = shape(key @ f32("Wk") + f32("bk"))
    q = shape(query @ f32("Wq") + f32("bq"))
    v = shape(value @ f32("Wv") + f32("bv"))
    efh = (sp(ef @ f32("ep_w1") + f32("ep_b1")) @ f32("ep_w2")
           + f32("ep_b2")).reshape(-1, H2, DH)
    bi, ii, jj = pi[0], pi[1], pi[2]
    kl = k[:, H2:].transpose(0, 2, 1, 3)[bi, jj] * efh
    ql = q[:, H2:].transpose(0, 2, 1, 3)[bi, ii] / math.sqrt(DH)
    sl = np.einsum("ehd,ehd->eh", ql, kl)
    se = np.full((B, N, N, H2), -1e18, np.float32)
    se[bi, ii, jj] = sl
    se = se.transpose(0, 3, 1, 2)
    m = se.max(axis=-1, keepdims=True)
    e = np.exp(se - m)
    al = e / e.sum(axis=-1, keepdims=True)
    al = np.where(se < -1e4, 0.0, al)
    local = al @ v[:, H2:]
    qg = q[:, :H2] / math.sqrt(DH)
    sg = np.einsum("bhqd,bhkd->bhqk", qg, k[:, :H2])
    top = sg[:, 0].copy()
    sg = np.where(mask[:, None], -1e18, sg)
    mg = sg.max(axis=-1, keepdims=True)
    eg = np.exp(sg - mg)
    ag = eg / eg.sum(axis=-1, keepdims=True)
    gctx = ag @ v[:, :H2]
    ctx = np.concatenate([gctx, local], axis=-1)
    ctx = ctx.transpose(0, 2, 1, 3).reshape(B, N, D)
    out = ctx @ f32("Wo") + f32("bo")
    nf = np.concatenate([out[bi, ii], out[bi, jj]], axis=-1)
    eupd = sp(nf @ f32("eu_w1") + f32("eu_b1")) @ f32("eu_w2") + f32("eu_b2")
    return out, top, eupd


def kernel(**inputs):
    if not _canonical(inputs["pair_indices"]):
        return _reference_numpy(inputs)

    f32 = lambda k: np.ascontiguousarray(np.asarray(inputs[k], np.float32))
    key, value, query = f32("key"), f32("value"), f32("query")
    mask = np.asarray(inputs["mask"])
    Wq, Wk, Wv, Wo = f32("Wq"), f32("Wk"), f32("Wv"), f32("Wo")
    bq, bk, bv, bo = f32("bq"), f32("bk"), f32("bv"), f32("bo")
    ep_w1, ep_b1 = f32("ep_w1"), f32("ep_b1")
    ep_w2, ep_b2 = f32("ep_w2"), f32("ep_b2")
    eu_w1, eu_b1 = f32("eu_w1"), f32("eu_b1")
    eu_w2, eu_b2 = f32("eu_w2"), f32("eu_b2")

    ep_b2a = ep_b2
    eu_b2a = eu_b2
    bq8 = bq * 0.125

    ef = np.asarray(inputs["edge_feature"], np.float32)
    efsm = np.ascontiguousarray(
        ef.reshape(B, N, DEG, D).transpose(0, 2, 3, 1).astype(_BF))
    invm = (~np.asarray(mask, bool)).astype(_BF)

    onesel, scidx, identf, identb = _host_constants()
    keyT = np.ascontiguousarray(key.transpose(0, 2, 1))
    queryT = np.ascontiguousarray(query.transpose(0, 2, 1))
    valueT = np.ascontiguousarray(value.transpose(0, 2, 1))

    shared = dict(
        wq=Wq, wk=Wk, wv=Wv, wo=Wo,
        epw1=ep_w1.astype(_BF), epw2=ep_w2.astype(_BF),
        euw1=eu_w1.astype(_BF), euw2=eu_w2.astype(_BF),
        bq8=bq8, bk=bk, bv=bv, bo=bo,
        epb1=ep_b1, epb2a=ep_b2a, eub1=eu_b1, eub2a=eu_b2a,
        onesel=onesel, scidx=scidx, identf=identf, identr=identf, identb=identb,
        onesr=np.ones((1, D), np.float32))

    in_maps = []
    for c in range(NCORES):
        bs = slice(c * NB, (c + 1) * NB)
        m = dict(shared)
        m["xk"] = keyT[bs]
        m["xq"] = queryT[bs]
        m["xv"] = valueT[bs]
        m["efsm"] = efsm[bs]
        m["invm"] = invm[bs]
        in_maps.append(m)

    nc = _build_program()
    import os as _os
    _trace = bool(_os.environ.get("BASS_TRACE"))
    r = run_bass_kernel_spmd(nc, in_maps, list(range(NCORES)), trace=_trace)
    _PROGRAM_CACHE["last_results"] = r

    out = np.concatenate([r.results[c]["out"] for c in range(NCORES)], axis=0)
    top = np.concatenate([r.results[c]["top"] for c in range(NCORES)], axis=0)
    eupd = np.concatenate([r.results[c]["eupd"] for c in range(NCORES)], axis=0)
    return out, top, eupd
